# revision 37
# baseline (speedup 1.0000x reference)
"""Trainium2 Bass kernel for nn_Kernel3D (Gaussian splat onto a 64x64x64x8 grid).

Math:  out[x,y,z,t] = sum_n bx[n,x] * by[n,y] * bz[n,z] * x[n,t]
where b?[n,g] = exp(-0.5*((g-mu)/s)^2) / sqrt(2*pi*s^2).

v4: x-slab sharding (8 planes/core) + z-sorted point chunks of 128 with a
uniform z-window width WZ (chunk z-offsets shared across cores so the SPMD
program is identical).  Per chunk the accumulated matmul is
    out[(x y), (z t)] += P[n, (x y)]^T @ Q[n, (z0..z0+wz) t]
Work is batched into large per-GROUP instructions and pipelined:

  DVE   d = g - mu; sqd = d*d (fp16 2x); usq = sqd * iv2-replica (2x);
        bz-replica over t via an int32-bitcast pair/quad copy (2x_2p);
        P and Q outer-product TTs at 2x
  ACT   one batched exp per group; fused broadcast-exp building the bx
        factor replicated along y (so P's TT runs at 2x); PSUM evacuation
  PE    4 zero-matmuls init PSUM; per chunk 4 blocks x (wz*8) fp16 columns
  GPSIMD memset + the single input DMA (cheap queue issue)

Host side: selection (slab dist <= C*sigma_x), z-sort, packing, window
metadata.  No host math on values.
"""

import os
import sys

import numpy as np

for _p in ("/opt/trn_rl_repo", "/root/.axon_site/_ro/trn_rl_repo"):
    if os.path.isdir(_p) and _p not in sys.path:
        sys.path.insert(0, _p)

N_CORES = 8
GX, GY, GZ, GT = 64, 64, 64, 8
XPER = GX // N_CORES
PPC = 128
FEAT = 16  # x[8], mu[3], sigma[3], pad[2]

SIGMA_CUT = 3.0
N_GROUPS = 3

_prog_cache = {}


def _build(n_chunks, z0s, z1s, g0s, WZ, ZS):
    import concourse.bass as bass
    import concourse.tile as tile
    from concourse import mybir
    from contextlib import ExitStack

    f32 = mybir.dt.float32
    f16 = mybir.dt.float16
    bf16 = mybir.dt.bfloat16
    u32 = mybir.dt.uint32
    AL = mybir.AluOpType
    ACTF = mybir.ActivationFunctionType
    C0 = float((2.0 * np.pi) ** -1.5)
    NC = n_chunks
    L = XPER + GY + WZ  # per-chunk flat segments [x | y | zwin]
    ZO = XPER + GY  # z segment offset
    HW = (ZS * GT, (GZ - ZS) * GT)  # used cols per (m,h) bank
    OH = (0, 4 * HW[0])  # o_t / out column offset of each half block

    # per-chunk (half, zlo, zhi) matmul parts; last chunk touching each half
    parts = []
    for c in range(NC):
        pr = []
        for h in (0, 1):
            zlo = max(z0s[c], ZS if h else 0)
            zhi = min(z1s[c], GZ if h else ZS)
            if zhi > zlo:
                pr.append((h, zlo, zhi))
        parts.append(pr)
    last_touch = {
        h: max(c for c in range(NC) if any(p[0] == h for p in parts[c]))
        for h in (0, 1)
    }

    G = min(N_GROUPS, NC)
    bounds = [round(g * NC / G) for g in range(G + 1)]
    groups = [(bounds[g], bounds[g + 1]) for g in range(G)]

    # one concatenated input: [pts (NC*16) | gx 8 | gy 64 | gz NC*WZ]
    TOT = NC * FEAT + XPER + GY + NC * WZ
    nc = bass.Bass(use_seq_codegen=True)
    inp = nc.declare_dram_parameter("inp", [PPC, TOT], f32, isOutput=False)
    out = nc.declare_dram_parameter("out", [PPC, GZ * GT * 4], bf16, isOutput=True)

    with tile.TileContext(nc) as tc, ExitStack() as ctx:
        cpool = ctx.enter_context(tc.tile_pool(name="const", bufs=1))
        ppool = ctx.enter_context(tc.tile_pool(name="accp", bufs=1, space="PSUM"))

        zero_t = cpool.tile([PPC, 640], f16, name="zero_t")
        nc.gpsimd.memset(zero_t[:, :], 0.0)
        dummy_t = cpool.tile([PPC, 1], f16, name="dummy_t")
        nc.scalar.activation(dummy_t[:, :], zero_t[:, 0:1], ACTF.Exp, scale=-0.5)

        inp_t = cpool.tile([PPC, TOT], f32, name="inp_t")
        NPTS = NC * FEAT
        GH = NPTS + (TOT - NPTS) // 2
        nc.sync.dma_start(inp_t[:, 0:NPTS], inp[:, 0:NPTS])  # pts land first
        nc.scalar.dma_start(inp_t[:, NPTS:GH], inp[:, NPTS:GH])
        nc.gpsimd.dma_start(inp_t[:, GH:TOT], inp[:, GH:TOT])
        pts3 = inp_t[:, 0 : NC * FEAT].rearrange("p (c f) -> p c f", f=FEAT)
        gx_t = inp_t[:, NC * FEAT : NC * FEAT + XPER]
        gy_t = inp_t[:, NC * FEAT + XPER : NC * FEAT + XPER + GY]
        gz3 = inp_t[:, NC * FEAT + XPER + GY : TOT].rearrange(
            "p (c w) -> p c w", w=WZ
        )

        # PSUM: 8 banks, bank (m, h) at cols (2m+h)*512; zero-matmul init
        acc = ppool.tile([128, 8 * 512], f32, name="acc")
        for m in range(4):
            for h in (0, 1):
                b = 2 * m + h
                nc.tensor.matmul(
                    acc[:, b * 512 : b * 512 + HW[h]],
                    lhsT=zero_t[:, 0:128],
                    rhs=zero_t[:, 128 : 128 + HW[h]],
                    start=True,
                    stop=False,
                )

        # per-point scalars
        inv_t = cpool.tile([PPC, NC, 3], f32, name="inv_t")
        nc.vector.reciprocal(inv_t[:, :, :], pts3[:, :, 11:14])
        iv2_t = cpool.tile([PPC, NC, 3], f16, name="iv2_t")
        nc.vector.tensor_tensor(iv2_t[:, :, :], inv_t[:, :, :], inv_t[:, :, :], AL.mult)
        ivzc_t = cpool.tile([PPC, NC], f32, name="ivzc_t")
        nc.vector.tensor_scalar(ivzc_t[:, :], inv_t[:, :, 2], C0, None, AL.mult)
        m1_t = cpool.tile([PPC, NC], f32, name="m1_t")
        nc.vector.tensor_tensor(m1_t[:, :], inv_t[:, :, 0], inv_t[:, :, 1], AL.mult)
        m2_t = cpool.tile([PPC, NC], f32, name="m2_t")
        nc.vector.tensor_tensor(m2_t[:, :], m1_t[:, :], ivzc_t[:, :], AL.mult)
        xc_t = cpool.tile([PPC, NC, GT], f16, name="xc_t")
        nc.vector.tensor_tensor(
            xc_t[:, :, :],
            pts3[:, :, 0:GT],
            m2_t[:, :].unsqueeze(2).broadcast_to((PPC, NC, GT)),
            AL.mult,
        )
        # iv2 replicated along the grid segments (fp16, 2x_2p copies)
        ivL_t = cpool.tile([PPC, NC, L], f16, name="ivL_t")
        nc.vector.tensor_copy(
            ivL_t[:, :, 0:XPER], iv2_t[:, :, 0:1].broadcast_to((PPC, NC, XPER))
        )
        nc.vector.tensor_copy(
            ivL_t[:, :, XPER:ZO], iv2_t[:, :, 1:2].broadcast_to((PPC, NC, GY))
        )
        nc.vector.tensor_copy(
            ivL_t[:, :, ZO:L], iv2_t[:, :, 2:3].broadcast_to((PPC, NC, WZ))
        )

        d_t = cpool.tile([PPC, NC, L], f16, name="d_t")
        sq_t = cpool.tile([PPC, NC, L], f16, name="sq_t")
        usq_t = cpool.tile([PPC, NC, L], f16, name="usq_t")
        b_t = cpool.tile([PPC, NC, L], f16, name="b_t")
        bxr_t = cpool.tile([PPC, NC, XPER, GY], f16, name="bxr_t")
        bzr_t = cpool.tile([PPC, NC, WZ, GT], f16, name="bzr_t")
        p_t = cpool.tile([PPC, NC, XPER, GY], f16, name="p_t")
        q_t = cpool.tile([PPC, NC, WZ, GT], f16, name="q_t")
        pf = p_t[:, :, :, :].rearrange("p c a b -> p c (a b)")
        qf = q_t[:, :, :, :].rearrange("p c a b -> p c (a b)")
        o_t = cpool.tile([128, GZ * GT * 4], bf16, name="o_t")

        def emit_evac(h):
            # evacuate the 4 (m,h) banks into a contiguous per-half block,
            # then ONE fat-descriptor DMA on the sync queue (gpsimd DMAs
            # stall the epilogue dge_drain; DVE copies mid-stream stall the
            # build pipeline, so half 0 runs entirely on ACT)
            W = HW[h]
            for m in range(4):
                b = 2 * m + h
                dst = o_t[:, OH[h] + m * W : OH[h] + (m + 1) * W]
                if h == 0 or m % 2 == 0:
                    nc.scalar.copy(dst, acc[:, b * 512 : b * 512 + W])
                else:
                    nc.vector.tensor_copy(dst, acc[:, b * 512 : b * 512 + W])
            cols = slice(OH[h], OH[h] + 4 * W)
            if h == 0:
                # mid-stream: sync ring only (keeps compute queues clean)
                nc.sync.dma_start(out[:, cols], o_t[:, cols])
            else:
                # end of kernel: scalar HWDGE + sync (after its small
                # half-0 transfer); avoid gpsimd SWDGE - its dge_drain
                # delays the teardown clears
                nc.scalar.dma_start(out[0:64, cols], o_t[0:64, cols])
                nc.sync.dma_start(out[64:128, cols], o_t[64:128, cols])

        def emit_front(g):
            c0, c1 = groups[g]
            n = c1 - c0
            nc.vector.tensor_tensor(
                d_t[:, c0:c1, 0:XPER],
                gx_t.unsqueeze(1).broadcast_to((PPC, n, XPER)),
                pts3[:, c0:c1, 8:9].broadcast_to((PPC, n, XPER)),
                AL.subtract,
            )
            nc.vector.tensor_tensor(
                d_t[:, c0:c1, XPER:ZO],
                gy_t.unsqueeze(1).broadcast_to((PPC, n, GY)),
                pts3[:, c0:c1, 9:10].broadcast_to((PPC, n, GY)),
                AL.subtract,
            )
            nc.vector.tensor_tensor(
                d_t[:, c0:c1, ZO:L],
                gz3[:, c0:c1, :],
                pts3[:, c0:c1, 10:11].broadcast_to((PPC, n, WZ)),
                AL.subtract,
            )
            nc.vector.tensor_tensor(
                sq_t[:, c0:c1, :], d_t[:, c0:c1, :], d_t[:, c0:c1, :], AL.mult
            )
            nc.vector.tensor_tensor(
                usq_t[:, c0:c1, :], sq_t[:, c0:c1, :], ivL_t[:, c0:c1, :], AL.mult
            )
            # ACT: compact exp + bx replicated along y via broadcast-exp
            nc.scalar.activation(
                b_t[:, c0:c1, :], usq_t[:, c0:c1, :], ACTF.Exp, scale=-0.5
            )
            nc.scalar.activation(
                bxr_t[:, c0:c1, :, :],
                usq_t[:, c0:c1, 0:XPER].unsqueeze(3).broadcast_to((PPC, n, XPER, GY)),
                ACTF.Exp,
                scale=-0.5,
            )

        def emit_back(g, last):
            c0, c1 = groups[g]
            n = c1 - c0
            # bz replicated over t: one 2x_2p broadcast copy
            nc.vector.tensor_copy(
                bzr_t[:, c0:c1, :, :],
                b_t[:, c0:c1, ZO:L].unsqueeze(3).broadcast_to((PPC, n, WZ, GT)),
            )
            nc.vector.tensor_tensor(
                q_t[:, c0:c1, :, :],
                bzr_t[:, c0:c1, :, :],
                xc_t[:, c0:c1, :].unsqueeze(2).broadcast_to((PPC, n, WZ, GT)),
                AL.mult,
            )
            nc.vector.tensor_tensor(
                p_t[:, c0:c1, :, :],
                bxr_t[:, c0:c1, :, :],
                b_t[:, c0:c1, XPER:ZO].unsqueeze(2).broadcast_to((PPC, n, XPER, GY)),
                AL.mult,
            )
            for c in range(c0, c1):
                for (h, zlo, zhi) in parts[c]:
                    s = zlo - g0s[c]
                    w = zhi - zlo
                    zb = ZS if h else 0
                    stop = c == last_touch[h]
                    for m in range(4):
                        b = 2 * m + h
                        nc.tensor.matmul(
                            acc[
                                :,
                                b * 512 + (zlo - zb) * GT : b * 512
                                + (zhi - zb) * GT,
                            ],
                            lhsT=pf[:, c, m * 128 : (m + 1) * 128],
                            rhs=qf[:, c, s * GT : (s + w) * GT],
                            start=False,
                            stop=stop and m == 3,
                        )
                if c == last_touch[0]:
                    emit_evac(0)

        emit_front(0)
        for g in range(1, G):
            emit_front(g)
            emit_back(g - 1, last=False)
        emit_back(G - 1, last=True)

        emit_evac(1)

    _split_multi_waits(nc, mybir)
    return nc


def _split_multi_waits(nc, mybir):
    k = 0
    for bb in nc.m.functions[0].blocks:
        new = []
        for inst in bb.instructions:
            si = inst.sync_info
            if si is not None and si.on_wait and len(si.on_wait) > 1:
                for w in si.on_wait[:-1]:
                    wi = mybir.InstEventSemaphore(name=f"wsplit_{k}", ins=[], outs=[])
                    k += 1
                    wi.engine = inst.engine
                    wi.sync_info = mybir.SyncInfo(on_wait=[w], on_update=[])
                    nc.register_instruction(wi)
                    new.append(wi)
                inst.sync_info = mybir.SyncInfo(
                    on_wait=[si.on_wait[-1]], on_update=si.on_update
                )
            new.append(inst)
        bb.instructions[:] = new


def _get_prog(n_chunks, z0s, z1s, g0s, WZ, ZS):
    key = (n_chunks, tuple(z0s), tuple(z1s), tuple(g0s), WZ, ZS, N_GROUPS, "v15")
    if key not in _prog_cache:
        _prog_cache[key] = _build(n_chunks, z0s, z1s, g0s, WZ, ZS)
    return _prog_cache[key]


def _pack_points(x, mu, sigma, n_chunks, z0s, wzs, core):
    n = x.shape[0]
    cap = n_chunks * PPC
    feat = np.zeros((cap, FEAT), np.float32)
    feat[:, 11:14] = 1.0
    for c in range(n_chunks):
        feat[c * PPC : (c + 1) * PPC, 8] = core * XPER + XPER / 2.0
        feat[c * PPC : (c + 1) * PPC, 9] = GY / 2.0
        feat[c * PPC : (c + 1) * PPC, 10] = z0s[c] + wzs[c] / 2.0
    feat[:n, 0:8] = x
    feat[:n, 8:11] = mu
    feat[:n, 11:14] = sigma
    return (
        feat.reshape(n_chunks, PPC, FEAT).transpose(1, 0, 2).reshape(PPC, n_chunks * FEAT)
    )


def _prepare(x, mu, sigma):
    n = x.shape[0]
    C = SIGMA_CUT
    sel = []
    for c in range(N_CORES):
        lo, hi = c * XPER, c * XPER + XPER - 1
        d = np.maximum.reduce([lo - mu[:, 0], mu[:, 0] - hi, np.zeros(n, np.float32)])
        idx = np.nonzero(d <= C * sigma[:, 0])[0]
        idx = idx[np.argsort(mu[idx, 2], kind="stable")]
        sel.append(idx)
    n_chunks = max(1, int(np.ceil(max(len(s) for s in sel) / PPC)))

    z0s, z1s = [], []
    for c in range(n_chunks):
        zlo, zhi = GZ, 0
        for k in range(N_CORES):
            idx = sel[k][c * PPC : (c + 1) * PPC]
            if len(idx):
                zlo = min(zlo, np.min(mu[idx, 2] - C * sigma[idx, 2]))
                zhi = max(zhi, np.max(mu[idx, 2] + C * sigma[idx, 2]))
        z0 = max(0, int(np.floor(zlo)))
        z1 = min(GZ, int(np.ceil(zhi)))
        if z1 <= z0:
            z0, z1 = 0, 1
        z0s.append(z0)
        z1s.append(z1)
    wzs = [z1s[c] - z0s[c] for c in range(n_chunks)]
    WZ = max(wzs)
    g0s = [z0s[c] if z0s[c] + WZ <= GZ else GZ - WZ for c in range(n_chunks)]
    # z-split so the low half of the output is final ~60% into the stream
    c_split = max(1, int(round(0.6 * n_chunks)))
    ZS = int(min(63, max(1, max(z0s[c] for c in range(c_split)) + 1)))

    gy = np.arange(GY, dtype=np.float32)
    in_maps = []
    for k in range(N_CORES):
        gz = np.concatenate(
            [np.arange(g0s[c], g0s[c] + WZ, dtype=np.float32) for c in range(n_chunks)]
        )
        idx = sel[k]
        row = np.concatenate(
            [
                np.zeros(n_chunks * FEAT, np.float32),  # pts placeholder
                np.arange(k * XPER, (k + 1) * XPER, dtype=np.float32),
                gy,
                gz,
            ]
        )
        inp = np.tile(row, (PPC, 1))
        inp[:, 0 : n_chunks * FEAT] = _pack_points(
            x[idx], mu[idx], sigma[idx], n_chunks, z0s, wzs, k
        )
        in_maps.append({"inp": inp})
    return in_maps, n_chunks, z0s, z1s, g0s, WZ, ZS


def _assemble(results, ZS):
    full = np.zeros((N_CORES, 512, GZ * GT), np.float32)
    W0, W1 = ZS * GT, (GZ - ZS) * GT
    for k in range(N_CORES):
        o = np.asarray(results[k]["out"]).astype(np.float32)  # [128, 2048] bf16
        for m in range(4):
            full[k, m * 128 : (m + 1) * 128, 0:W0] = o[:, m * W0 : (m + 1) * W0]
            full[k, m * 128 : (m + 1) * 128, W0:] = o[
                :, 4 * W0 + m * W1 : 4 * W0 + (m + 1) * W1
            ]
    o = full.reshape(N_CORES, XPER, GY, GZ, GT)
    return np.ascontiguousarray(o.reshape(GX, GY, GZ, GT))


def run(x, mu, sigma, trace=False, **spmd_kwargs):
    from concourse.bass_utils import run_bass_kernel_spmd

    x = np.asarray(x, np.float32)
    mu = np.asarray(mu, np.float32)
    sigma = np.asarray(sigma, np.float32)
    in_maps, n_chunks, z0s, z1s, g0s, WZ, ZS = _prepare(x, mu, sigma)
    nc = _get_prog(n_chunks, z0s, z1s, g0s, WZ, ZS)
    res = run_bass_kernel_spmd(
        nc, in_maps, list(range(N_CORES)), trace=trace, **spmd_kwargs
    )
    return _assemble(res.results, ZS), res


def kernel(x, mu, sigma):
    out, _ = run(x, mu, sigma)
    return out
